# revision 35
# baseline (speedup 1.0000x reference)
"""YOLO-style loss (nn_Loss_90142773608781) on 8 Trainium2 NeuronCores.

Strategy (data-parallel by cell range, per sharding hint):
- Cells (16384*7*7 = 802816 rows of 30 floats) are sharded by batch range:
  core c owns cells [c*100352, (c+1)*100352).
- Dense conf term: host extracts cols {4,9} to a [CELLS,2] array; each core
  squares+accumulates its slice on ScalarE (one op).
- Targeted terms: grid rows are fetched with chunked dma_gather ops
  (single-packet SWDGE, 64 descs/engine packet ceiling) from an f32 table
  padded to 128B rows: each 256B gathered element covers 2 cells. Per core
  the cell range is split into 2 windows (int16 index reach) x cell parity
  -> 4 fixed-capacity slot groups, so each slot's sub-offset is
  compile-time. Queue-ring reuse has a fixed ~8.7us recycle, so the 10
  chunks are issued in 3 rounds over all 4 SWDGE queues. The math runs in
  4 sections (one per slot group, reading the gather tile through strided
  views - no repacking) pipelined against the drains.
- Every reduction runs as a ScalarE Square+accumulate: 5*(coord/size
  diff)^2 via scale=sqrt(5); (cr-1)^2-0.5cr^2 = 0.5(cr-2)^2-1 via
  scale=1/sqrt(2), bias=-sqrt(2); sum(cls^2) directly; and the class-hit
  term -2*cls_sel via the polarization identity sum(h*cls) =
  (sum((h+cls)^2) - sum(h^2) - sum(cls^2))/2 with h = -2*onehot baked on
  the host (sum(h^2) = 4*NTGT globally). The DVE only does elementwise
  work, never reduces.
- Padding slots gather a dedicated all-zero row; their only residue,
  0.5*(0-2)^2 = 2 per slot, is corrected on the host.
- Each core writes [128,17] partials; host reduces and applies the
  constant corrections.
"""

import sys

if "/opt/trn_rl_repo" not in sys.path:
    sys.path.append("/opt/trn_rl_repo")

import numpy as np

P = 128
D = 30
GRID = 7
BATCH = 16384
NTGT = 65536
CELLS = BATCH * GRID * GRID          # 802816
CELLS_CORE = CELLS // 8              # 100352
WCELLS = CELLS_CORE // 2             # 50176 cells per window
PR_WIN = WCELLS // 2                 # 25088 pair rows per window
ZROW = PR_WIN                        # dedicated zero row per window
WINROWS = PR_WIN + 1                 # 25089
CAP = 2304                           # slots per (window, parity) group
NG = 4
NS = CAP * NG                        # 9216 slots per core
NCHUNK = NS // P                     # 72
GC = CAP // P                        # 18 chunks per group
IDXW = NS // 16                      # 576
CONF_W = CELLS_CORE * 2 // P         # 1568

# gather chunks: (slot0, nslots, window, queue). Three rounds over the 4
# SWDGE queues; the small 512 chunks absorb the odd round slots.
GATHERS = [
    (0, 1024, 0, 0), (1024, 1024, 0, 1), (2048, 1024, 0, 2),
    (3072, 1024, 0, 3),
    (4096, 512, 0, 0), (4608, 1024, 1, 1), (5632, 1024, 1, 2),
    (6656, 1024, 1, 3),
    (7680, 1024, 1, 0), (8704, 512, 1, 1),
]

RT5 = 2.2360679774997896         # sqrt(5)
RT2I = 0.7071067811865476        # 1/sqrt(2)
RT2 = 1.4142135623730951         # sqrt(2)

_cache = {}


def _build():
    import concourse.bacc as bacc
    import concourse.tile as tile
    import concourse.mybir as mybir
    from concourse import library_config

    F32 = mybir.dt.float32
    I16 = mybir.dt.int16
    AL = mybir.AluOpType
    ACT = mybir.ActivationFunctionType
    X = mybir.AxisListType.X

    nc = bacc.Bacc("TRN2", target_bir_lowering=False, debug=False,
                   enable_asserts=False, num_devices=8, num_swdge_queues=4,
                   dynamic_dma_scratch_size=2 ** 16)
    win = nc.dram_tensor("win", [2 * WINROWS, 64], F32,
                         kind="ExternalInput").ap()
    idx = nc.dram_tensor("idx", [P, IDXW], I16, kind="ExternalInput").ap()
    fldf = nc.dram_tensor("fldf", [P, 9 * NCHUNK], F32,
                          kind="ExternalInput").ap()
    hcls = nc.dram_tensor("hcls", [P, 20 * NCHUNK], F32,
                          kind="ExternalInput").ap()
    conf = nc.dram_tensor("conf", [P, CONF_W], F32, kind="ExternalInput").ap()
    out = nc.dram_tensor("partial", [P, 17], F32, kind="ExternalOutput").ap()

    vec, act = nc.vector, nc.scalar

    with tile.TileContext(nc) as tc:
        with (
            tc.tile_pool(name="io", bufs=1) as io,
            tc.tile_pool(name="scr", bufs=2) as scr,
        ):
            # preload the ucode library containing DMAGatherAnt so its IRAM
            # load overlaps the input DMAs instead of gating the first gather
            nc.gpsimd.load_library(library_config.mlp)

            # ---- idx loads: one slice per gather chunk, on the ACT HWDGE
            # ring so they don't queue behind the big sync-engine loads ----
            idx_t = io.tile([P, IDXW], I16)
            for (n0, n, w, q) in GATHERS:
                c0, c1 = n0 // 16, (n0 + n) // 16
                nc.scalar.dma_start(out=idx_t[:, c0:c1], in_=idx[:, c0:c1])

            eps_t = io.tile([P, 1], F32)
            vec.memset(eps_t[:], 1e-6)
            nrt2_t = io.tile([P, 1], F32)
            vec.memset(nrt2_t[:], -RT2)
            # ---- main gathers ----
            g = io.tile([P, NS * 64 // P], F32)
            g3v = g[:].rearrange("p (k e) -> p k e", e=64)
            for (n0, n, w, q) in GATHERS:
                nc.gpsimd.dma_gather(
                    g3v[:, n0 // P:(n0 + n) // P, :],
                    win[w * WINROWS:(w + 1) * WINROWS, :],
                    idx_t[:, n0 // 16:(n0 + n) // 16], n, n, 64,
                    queue_num=q,
                )

            # ---- other loads ----
            conf_t = io.tile([P, CONF_W], F32)
            nc.sync.dma_start(out=conf_t[:], in_=conf[:])
            fld_t = io.tile([P, 9 * NCHUNK], F32)
            nc.sync.dma_start(out=fld_t[:], in_=fldf[:])
            h_t = io.tile([P, 20 * NCHUNK], F32)
            nc.sync.dma_start(out=h_t[:], in_=hcls[:])

            acc = io.tile([P, 17], F32)

            # ---- dense conf term on ScalarE: accum 0.5*conf^2 ----
            confsq = scr.tile([P, CONF_W], F32, tag="confsq")
            act.activation(confsq[:], conf_t[:], ACT.Square, scale=RT2I,
                           accum_out=acc[:, 16:17])

            # field views: [xyssq 4-wide][lt 2][rb 2][area 1]
            xys_all = fld_t[:, :4 * NCHUNK].rearrange("p (k c) -> p k c", c=4)
            lt_all = fld_t[:, 4 * NCHUNK:6 * NCHUNK].rearrange(
                "p (k c) -> p k c", c=2)
            rb_all = fld_t[:, 6 * NCHUNK:8 * NCHUNK].rearrange(
                "p (k c) -> p k c", c=2)
            area_all = fld_t[:, 8 * NCHUNK:9 * NCHUNK]
            h3_all = h_t[:].rearrange("p (k c) -> p k c", c=20)

            def sec_math(si):
                k0, k1 = si * GC, (si + 1) * GC
                W = GC
                m = si % 2
                sfx = str(si)
                # strided views straight into the gathered tile
                gsec = g[:, k0 * 64 + 32 * m:]          # offset view base
                g5 = g3v[:, k0:k1, 32 * m:32 * m + D].rearrange(
                    "p k (b r) -> p k b r", r=5)
                xy = g5[:, :, 0:2, 0:2]
                wh = g5[:, :, 0:2, 2:4]
                clsg = g3v[:, k0:k1, 32 * m + 10:32 * m + 30]
                XYS4 = xys_all[:, k0:k1, :]
                LTt = lt_all[:, k0:k1, :].unsqueeze(2).to_broadcast(
                    [P, W, 2, 2])
                RBt = rb_all[:, k0:k1, :].unsqueeze(2).to_broadcast(
                    [P, W, 2, 2])
                areab = area_all[:, k0:k1].unsqueeze(2).to_broadcast([P, W, 2])
                h3 = h3_all[:, k0:k1, :]

                def t4(tag):
                    t = scr.tile([P, W * 4], F32, tag=tag + sfx, name=tag + sfx)
                    return t[:].rearrange("p (k b r) -> p k b r", b=2, r=2)

                def t2(tag):
                    t = scr.tile([P, W * 2], F32, tag=tag + sfx, name=tag + sfx)
                    return t[:].rearrange("p (k c) -> p k c", c=2)

                def t1(tag):
                    return scr.tile([P, W], F32, tag=tag + sfx,
                                    name=tag + sfx)[:]

                hwh = t4("hwh")
                vec.tensor_scalar_mul(out=hwh, in0=wh, scalar1=3.5)
                lt = t4("lt")
                vec.tensor_tensor(out=lt, in0=xy, in1=hwh, op=AL.subtract)
                rb = t4("rb")
                vec.tensor_tensor(out=rb, in0=xy, in1=hwh, op=AL.add)

                wih = t4("wih")
                vec.tensor_tensor(out=wih, in0=rb, in1=RBt, op=AL.min)
                mx = t4("mx")
                vec.tensor_tensor(out=mx, in0=lt, in1=LTt, op=AL.max)
                vec.tensor_tensor(out=wih, in0=wih, in1=mx, op=AL.subtract)
                vec.tensor_scalar_max(out=wih, in0=wih, scalar1=0.0)

                ain = t2("ain")
                vec.tensor_tensor(out=ain, in0=wih[:, :, :, 0],
                                  in1=wih[:, :, :, 1], op=AL.mult)
                atot = t2("atot")
                vec.tensor_tensor(out=atot, in0=wh[:, :, :, 0],
                                  in1=wh[:, :, :, 1], op=AL.mult)
                vec.tensor_scalar_mul(out=atot, in0=atot, scalar1=49.0)
                vec.tensor_tensor(out=atot, in0=atot, in1=areab, op=AL.add)
                vec.tensor_tensor(out=atot, in0=atot, in1=ain, op=AL.subtract)

                # sel = iou1 > iou0 via cross-multiply: u=max(atot,eps)>0,
                # v=ain*(atot>eps)>=0 -> sel <=> v1*u0 > v0*u1.
                pred = t2("pred")
                vec.tensor_scalar(out=pred, in0=atot, scalar1=1e-6,
                                  scalar2=None, op0=AL.is_gt)
                vec.tensor_scalar_max(out=atot, in0=atot, scalar1=1e-6)
                vec.tensor_tensor(out=pred, in0=ain, in1=pred, op=AL.mult)
                c10 = t1("c10")
                vec.tensor_tensor(out=c10, in0=pred[:, :, 1],
                                  in1=atot[:, :, 0], op=AL.mult)
                c01 = t1("c01")
                vec.tensor_tensor(out=c01, in0=pred[:, :, 0],
                                  in1=atot[:, :, 1], op=AL.mult)
                sel1 = t1("sel1")
                vec.tensor_tensor(out=sel1, in0=c10, in1=c01, op=AL.is_gt)
                sel5 = sel1.unsqueeze(2).to_broadcast([P, W, 5])

                # 5-wide responsible-box pick: r = b0 + sel*(b1-b0)
                t5t = scr.tile([P, W * 5], F32, tag="t5" + sfx, name="t5" + sfx)
                t5 = t5t[:].rearrange("p (k c) -> p k c", c=5)
                vec.tensor_tensor(out=t5, in0=g5[:, :, 1, 0:5],
                                  in1=g5[:, :, 0, 0:5], op=AL.subtract)
                vec.tensor_tensor(out=t5, in0=t5, in1=sel5, op=AL.mult)
                vec.tensor_tensor(out=t5, in0=t5, in1=g5[:, :, 0, 0:5],
                                  op=AL.add)
                whr = t5[:, :, 2:4]
                cr = t5[:, :, 4]

                # signed sqrt of whr written back over whr -> t5[:, :, 0:4]
                # becomes (x_r, y_r, ssq(w_r), ssq(h_r))
                sq_ = t2("sq_")
                sg_ = t2("sg_")
                act.activation(sq_, whr, ACT.Abs)
                act.activation(sq_, sq_, ACT.Sqrt, bias=eps_t[:])
                act.activation(sg_, whr, ACT.Sign)
                vec.tensor_tensor(out=t5[:, :, 2:4], in0=sq_, in1=sg_,
                                  op=AL.mult)

                # coord+size: accum 5*sum((XYS4 - t5[:,:,0:4])^2)
                d4 = t4("d4")
                d4f = d4.rearrange("p k b r -> p k (b r)")
                vec.tensor_tensor(out=d4f, in0=XYS4, in1=t5[:, :, 0:4],
                                  op=AL.subtract)
                d4sq = t4("d4sq")
                act.activation(d4sq.rearrange("p k b r -> p k (b r)"), d4f,
                               ACT.Square, scale=RT5,
                               accum_out=acc[:, si:si + 1])

                # obj: accum 0.5*(cr-2)^2  (= (cr-1)^2 - 0.5cr^2 + 1)
                o1 = t1("o1")
                act.activation(o1, cr, ACT.Square, scale=RT2I, bias=nrt2_t[:],
                               accum_out=acc[:, 4 + si:5 + si])

                # class terms: accum sum(cls^2) and sum((h+cls)^2)
                clssq = scr.tile([P, W * 20], F32, tag="clssq" + sfx,
                                 name="clssq" + sfx)
                act.activation(clssq[:].rearrange("p (k c) -> p k c", c=20),
                               clsg, ACT.Square,
                               accum_out=acc[:, 8 + si:9 + si])
                big = scr.tile([P, W * 20], F32, tag="big" + sfx,
                               name="big" + sfx)
                big3 = big[:].rearrange("p (k c) -> p k c", c=20)
                vec.tensor_tensor(out=big3, in0=h3, in1=clsg, op=AL.add)
                hpc = scr.tile([P, W * 20], F32, tag="hpc" + sfx,
                               name="hpc" + sfx)
                act.activation(hpc[:].rearrange("p (k c) -> p k c", c=20),
                               big3, ACT.Square,
                               accum_out=acc[:, 12 + si:13 + si])

            for si in range(NG):
                sec_math(si)

            nc.sync.dma_start(out=out[:], in_=acc[:])
    nc.compile()
    return nc


def _get_nc():
    if "nc" not in _cache:
        _cache["nc"] = _build()
    return _cache["nc"]


def _host_prep(output, target):
    f32 = np.float32
    out_flat = output.reshape(CELLS, D)

    pt = np.zeros((CELLS, 32), dtype=f32)
    pt[:, :D] = out_flat
    conf_all = np.ascontiguousarray(out_flat[:, 4:10:5])

    bid = target[:, 7].astype(np.int64)
    gx = target[:, 4].astype(np.int64)
    gy = target[:, 5].astype(np.int64)
    cell = bid * (GRID * GRID) + gx * GRID + gy

    order = np.argsort(cell, kind="stable")
    ts = target[order]
    cs = cell[order]
    core = cs // CELLS_CORE
    wloc = (cs % CELLS_CORE) // WCELLS           # window 0/1
    mod = cs % 2                                 # parity within pair row
    grp = wloc * 2 + mod
    lp = ((cs % WCELLS) // 2).astype(np.int16)   # local pair row [0, 25088)

    x = ts[:, 0].astype(f32)
    y = ts[:, 1].astype(f32)
    w_ = ts[:, 2].astype(f32)
    h_ = ts[:, 3].astype(f32)
    c35 = f32(3.5)
    fields = np.empty((NTGT, 9), dtype=f32)
    fields[:, 0] = x
    fields[:, 1] = y
    fields[:, 2] = np.sign(w_) * np.sqrt(np.abs(w_) + f32(1e-6))   # ssqw
    fields[:, 3] = np.sign(h_) * np.sqrt(np.abs(h_) + f32(1e-6))   # ssqh
    fields[:, 4] = x - c35 * w_      # lef
    fields[:, 5] = y - c35 * h_      # top
    fields[:, 6] = x + c35 * w_      # rig
    fields[:, 7] = y + c35 * h_      # bot
    fields[:, 8] = (w_ * h_) * f32(49.0)
    clsid = ts[:, 6].astype(np.int64)
    hoh_all = np.zeros((NTGT, 20), dtype=f32)
    hoh_all[np.arange(NTGT), clsid] = f32(-2.0)

    in_maps = []
    for c in range(8):
        sel_c = core == c
        idxs = np.full(NS, ZROW, dtype=np.int16)
        fld = np.zeros((NS, 9), dtype=f32)
        hoh = np.zeros((NS, 20), dtype=f32)
        for gi in range(NG):
            selm = sel_c & (grp == gi)
            n = int(selm.sum())
            assert n <= CAP, f"group overflow: core {c} grp {gi} n={n}"
            s0 = gi * CAP
            idxs[s0:s0 + n] = lp[selm]
            fld[s0:s0 + n] = fields[selm]
            hoh[s0:s0 + n] = hoh_all[selm]

        idx16 = np.tile(idxs.reshape(IDXW, 16).T, (8, 1))          # [128, 576]
        fldf = np.empty((P, 9 * NCHUNK), dtype=f32)
        fldf[:, :4 * NCHUNK] = fld[:, 0:4].reshape(
            NCHUNK, P, 4).transpose(1, 0, 2).reshape(P, 4 * NCHUNK)
        fldf[:, 4 * NCHUNK:6 * NCHUNK] = fld[:, 4:6].reshape(
            NCHUNK, P, 2).transpose(1, 0, 2).reshape(P, 2 * NCHUNK)
        fldf[:, 6 * NCHUNK:8 * NCHUNK] = fld[:, 6:8].reshape(
            NCHUNK, P, 2).transpose(1, 0, 2).reshape(P, 2 * NCHUNK)
        fldf[:, 8 * NCHUNK:] = fld[:, 8].reshape(NCHUNK, P).T
        hcls = np.ascontiguousarray(
            hoh.reshape(NCHUNK, P, 20).transpose(1, 0, 2).reshape(P, 20 * NCHUNK))
        wslab = pt[c * CELLS_CORE:(c + 1) * CELLS_CORE].reshape(2, PR_WIN, 64)
        win = np.zeros((2 * WINROWS, 64), dtype=f32)
        win[:PR_WIN] = wslab[0]
        win[WINROWS:WINROWS + PR_WIN] = wslab[1]
        confc = np.ascontiguousarray(
            conf_all[c * CELLS_CORE:(c + 1) * CELLS_CORE]).reshape(P, CONF_W)
        in_maps.append({
            "win": win,
            "idx": np.ascontiguousarray(idx16),
            "fldf": fldf,
            "hcls": hcls,
            "conf": confc,
        })
    return in_maps


def _reduce(results):
    # cols 0-3: 5*coordsize; 4-7: 0.5*(cr-2)^2; 8-11: S_c = sum(cls^2);
    # 12-15: S_hpc = sum((h+cls)^2); 16: 0.5*conf^2.
    # loss = coordsize + obj + S_c + (S_hpc - S_c - 4*NTGT)/2 + conf
    #        - 2*n_pad (obj residue of padding) - 0 (real-slot +1's cancel)
    tot = 0.0
    for res in results:
        p = res["partial"].astype(np.float64)
        tot += float(p[:, 0:8].sum())                       # coordsize + obj
        tot += float((p[:, 8:12] + p[:, 12:16]).sum()) / 2  # (S_c + S_hpc)/2
        tot += float(p[:, 16].sum())                        # conf
    tot -= 2.0 * NTGT                  # polarization: -4*NTGT/2
    tot -= 2.0 * (8 * NS - NTGT)       # padding obj residue
    return np.float32(tot)


def run(output, target, trace=False, trace_cores=None):
    from concourse.bass_utils import run_bass_kernel_spmd

    nc = _get_nc()
    in_maps = _host_prep(np.asarray(output), np.asarray(target))
    r = run_bass_kernel_spmd(nc, in_maps, core_ids=list(range(8)), trace=trace,
                             trace_cores=trace_cores)
    return _reduce(r.results), r


def kernel(output, target):
    return run(output, target)[0]


# revision 36
# speedup vs baseline: 1.0073x; 1.0073x over previous
"""YOLO-style loss (nn_Loss_90142773608781) on 8 Trainium2 NeuronCores.

Strategy (data-parallel by cell range, per sharding hint):
- Cells (16384*7*7 = 802816 rows of 30 floats) are sharded by batch range:
  core c owns cells [c*100352, (c+1)*100352).
- Dense conf term: host extracts cols {4,9} to a [CELLS,2] array; each core
  squares+accumulates its slice on ScalarE (one op).
- Targeted terms: grid rows are fetched with chunked dma_gather ops
  (single-packet SWDGE, 64 descs/engine packet ceiling) from an f32 table
  padded to 128B rows: each 256B gathered element covers 2 cells. Per core
  the cell range is split into 2 windows (int16 index reach) x cell parity
  -> 4 fixed-capacity slot groups, so each slot's sub-offset is
  compile-time. Queue-ring reuse has a fixed ~8.7us recycle, so the 10
  chunks are issued in 3 rounds over all 4 SWDGE queues. The math runs in
  4 sections (one per slot group, reading the gather tile through strided
  views - no repacking) pipelined against the drains.
- Every reduction runs as a ScalarE Square+accumulate: 5*(coord/size
  diff)^2 via scale=sqrt(5); (cr-1)^2-0.5cr^2 = 0.5(cr-2)^2-1 via
  scale=1/sqrt(2), bias=-sqrt(2); sum(cls^2) directly; and the class-hit
  term -2*cls_sel via the polarization identity sum(h*cls) =
  (sum((h+cls)^2) - sum(h^2) - sum(cls^2))/2 with h = -2*onehot baked on
  the host (sum(h^2) = 4*NTGT globally). The DVE only does elementwise
  work, never reduces.
- Padding slots gather a dedicated all-zero row; their only residue,
  0.5*(0-2)^2 = 2 per slot, is corrected on the host.
- Each core writes [128,17] partials; host reduces and applies the
  constant corrections.
"""

import sys

if "/opt/trn_rl_repo" not in sys.path:
    sys.path.append("/opt/trn_rl_repo")

import numpy as np

P = 128
D = 30
GRID = 7
BATCH = 16384
NTGT = 65536
CELLS = BATCH * GRID * GRID          # 802816
CELLS_CORE = CELLS // 8              # 100352
WCELLS = CELLS_CORE // 2             # 50176 cells per window
PR_WIN = WCELLS // 2                 # 25088 pair rows per window
ZROW = PR_WIN                        # dedicated zero row per window
WINROWS = PR_WIN + 1                 # 25089
CAP = 2304                           # slots per (window, parity) group
NG = 4
NS = CAP * NG                        # 9216 slots per core
NCHUNK = NS // P                     # 72
GC = CAP // P                        # 18 chunks per group
IDXW = NS // 16                      # 576
CONF_W = CELLS_CORE * 2 // P         # 1568

# gather chunks: (slot0, nslots, window, queue). Three rounds over the 4
# SWDGE queues; the small 512 chunks absorb the odd round slots.
GATHERS = [
    (0, 1024, 0, 0), (1024, 1024, 0, 1), (2048, 1024, 0, 2),
    (3072, 1024, 0, 3),
    (4096, 512, 0, 0), (4608, 1024, 1, 1), (5632, 1024, 1, 2),
    (6656, 1024, 1, 3),
    (7680, 1024, 1, 0), (8704, 512, 1, 1),
]

RT5 = 2.2360679774997896         # sqrt(5)
RT2I = 0.7071067811865476        # 1/sqrt(2)
RT2 = 1.4142135623730951         # sqrt(2)

_cache = {}


def _build():
    import concourse.bacc as bacc
    import concourse.tile as tile
    import concourse.mybir as mybir
    from concourse import library_config

    F32 = mybir.dt.float32
    I16 = mybir.dt.int16
    AL = mybir.AluOpType
    ACT = mybir.ActivationFunctionType
    X = mybir.AxisListType.X

    nc = bacc.Bacc("TRN2", target_bir_lowering=False, debug=False,
                   enable_asserts=False, num_devices=8, num_swdge_queues=4,
                   dynamic_dma_scratch_size=2 ** 16)
    win = nc.dram_tensor("win", [2 * WINROWS, 64], F32,
                         kind="ExternalInput").ap()
    idx = nc.dram_tensor("idx", [P, IDXW], I16, kind="ExternalInput").ap()
    fldf = nc.dram_tensor("fldf", [P, 9 * NCHUNK], F32,
                          kind="ExternalInput").ap()
    hcls = nc.dram_tensor("hcls", [P, 20 * NCHUNK], F32,
                          kind="ExternalInput").ap()
    conf = nc.dram_tensor("conf", [P, CONF_W], F32, kind="ExternalInput").ap()
    out = nc.dram_tensor("partial", [P, 17], F32, kind="ExternalOutput").ap()

    vec, act = nc.vector, nc.scalar

    with tile.TileContext(nc) as tc:
        with (
            tc.tile_pool(name="io", bufs=1) as io,
            tc.tile_pool(name="scr", bufs=2) as scr,
        ):
            # preload the ucode library containing DMAGatherAnt so its IRAM
            # load overlaps the input DMAs instead of gating the first gather
            nc.gpsimd.load_library(library_config.mlp)

            # ---- idx loads: one slice per gather chunk, on the ACT HWDGE
            # ring so they don't queue behind the big sync-engine loads ----
            idx_t = io.tile([P, IDXW], I16)
            for (n0, n, w, q) in GATHERS:
                c0, c1 = n0 // 16, (n0 + n) // 16
                nc.scalar.dma_start(out=idx_t[:, c0:c1], in_=idx[:, c0:c1])

            eps_t = io.tile([P, 1], F32)
            vec.memset(eps_t[:], 1e-6)
            nrt2_t = io.tile([P, 1], F32)
            vec.memset(nrt2_t[:], -RT2)
            # dummy activations: force the ACT function-table loads into the
            # startup window instead of mid-math
            dum = scr.tile([P, 1], F32, tag="dum")
            act.activation(dum[:], eps_t[:], ACT.Square)
            act.activation(dum[:], eps_t[:], ACT.Abs)
            act.activation(dum[:], eps_t[:], ACT.Sqrt)
            act.activation(dum[:], eps_t[:], ACT.Sign)

            # ---- main gathers ----
            g = io.tile([P, NS * 64 // P], F32)
            g3v = g[:].rearrange("p (k e) -> p k e", e=64)
            for (n0, n, w, q) in GATHERS:
                nc.gpsimd.dma_gather(
                    g3v[:, n0 // P:(n0 + n) // P, :],
                    win[w * WINROWS:(w + 1) * WINROWS, :],
                    idx_t[:, n0 // 16:(n0 + n) // 16], n, n, 64,
                    queue_num=q,
                )

            # ---- other loads ----
            conf_t = io.tile([P, CONF_W], F32)
            nc.sync.dma_start(out=conf_t[:], in_=conf[:])
            fld_t = io.tile([P, 9 * NCHUNK], F32)
            nc.sync.dma_start(out=fld_t[:], in_=fldf[:])
            h_t = io.tile([P, 20 * NCHUNK], F32)
            nc.sync.dma_start(out=h_t[:], in_=hcls[:])

            acc = io.tile([P, 17], F32)

            # ---- dense conf term on ScalarE: accum 0.5*conf^2 ----
            confsq = scr.tile([P, CONF_W], F32, tag="confsq")
            act.activation(confsq[:], conf_t[:], ACT.Square, scale=RT2I,
                           accum_out=acc[:, 16:17])

            # field views: [xyssq 4-wide][lt 2][rb 2][area 1]
            xys_all = fld_t[:, :4 * NCHUNK].rearrange("p (k c) -> p k c", c=4)
            lt_all = fld_t[:, 4 * NCHUNK:6 * NCHUNK].rearrange(
                "p (k c) -> p k c", c=2)
            rb_all = fld_t[:, 6 * NCHUNK:8 * NCHUNK].rearrange(
                "p (k c) -> p k c", c=2)
            area_all = fld_t[:, 8 * NCHUNK:9 * NCHUNK]
            h3_all = h_t[:].rearrange("p (k c) -> p k c", c=20)

            def sec_math(si):
                k0, k1 = si * GC, (si + 1) * GC
                W = GC
                m = si % 2
                sfx = str(si)
                # strided views straight into the gathered tile
                gsec = g[:, k0 * 64 + 32 * m:]          # offset view base
                g5 = g3v[:, k0:k1, 32 * m:32 * m + D].rearrange(
                    "p k (b r) -> p k b r", r=5)
                xy = g5[:, :, 0:2, 0:2]
                wh = g5[:, :, 0:2, 2:4]
                clsg = g3v[:, k0:k1, 32 * m + 10:32 * m + 30]
                XYS4 = xys_all[:, k0:k1, :]
                LTt = lt_all[:, k0:k1, :].unsqueeze(2).to_broadcast(
                    [P, W, 2, 2])
                RBt = rb_all[:, k0:k1, :].unsqueeze(2).to_broadcast(
                    [P, W, 2, 2])
                areab = area_all[:, k0:k1].unsqueeze(2).to_broadcast([P, W, 2])
                h3 = h3_all[:, k0:k1, :]

                def t4(tag):
                    t = scr.tile([P, W * 4], F32, tag=tag + sfx, name=tag + sfx)
                    return t[:].rearrange("p (k b r) -> p k b r", b=2, r=2)

                def t2(tag):
                    t = scr.tile([P, W * 2], F32, tag=tag + sfx, name=tag + sfx)
                    return t[:].rearrange("p (k c) -> p k c", c=2)

                def t1(tag):
                    return scr.tile([P, W], F32, tag=tag + sfx,
                                    name=tag + sfx)[:]

                hwh = t4("hwh")
                vec.tensor_scalar_mul(out=hwh, in0=wh, scalar1=3.5)
                lt = t4("lt")
                vec.tensor_tensor(out=lt, in0=xy, in1=hwh, op=AL.subtract)
                rb = t4("rb")
                vec.tensor_tensor(out=rb, in0=xy, in1=hwh, op=AL.add)

                wih = t4("wih")
                vec.tensor_tensor(out=wih, in0=rb, in1=RBt, op=AL.min)
                mx = t4("mx")
                vec.tensor_tensor(out=mx, in0=lt, in1=LTt, op=AL.max)
                vec.tensor_tensor(out=wih, in0=wih, in1=mx, op=AL.subtract)
                vec.tensor_scalar_max(out=wih, in0=wih, scalar1=0.0)

                ain = t2("ain")
                vec.tensor_tensor(out=ain, in0=wih[:, :, :, 0],
                                  in1=wih[:, :, :, 1], op=AL.mult)
                atot = t2("atot")
                vec.tensor_tensor(out=atot, in0=wh[:, :, :, 0],
                                  in1=wh[:, :, :, 1], op=AL.mult)
                vec.tensor_scalar_mul(out=atot, in0=atot, scalar1=49.0)
                vec.tensor_tensor(out=atot, in0=atot, in1=areab, op=AL.add)
                vec.tensor_tensor(out=atot, in0=atot, in1=ain, op=AL.subtract)

                # sel = iou1 > iou0 via cross-multiply: u=max(atot,eps)>0,
                # v=ain*(atot>eps)>=0 -> sel <=> v1*u0 > v0*u1.
                pred = t2("pred")
                vec.tensor_scalar(out=pred, in0=atot, scalar1=1e-6,
                                  scalar2=None, op0=AL.is_gt)
                vec.tensor_scalar_max(out=atot, in0=atot, scalar1=1e-6)
                vec.tensor_tensor(out=pred, in0=ain, in1=pred, op=AL.mult)
                c10 = t1("c10")
                vec.tensor_tensor(out=c10, in0=pred[:, :, 1],
                                  in1=atot[:, :, 0], op=AL.mult)
                c01 = t1("c01")
                vec.tensor_tensor(out=c01, in0=pred[:, :, 0],
                                  in1=atot[:, :, 1], op=AL.mult)
                sel1 = t1("sel1")
                vec.tensor_tensor(out=sel1, in0=c10, in1=c01, op=AL.is_gt)
                sel5 = sel1.unsqueeze(2).to_broadcast([P, W, 5])

                # 5-wide responsible-box pick: r = b0 + sel*(b1-b0)
                t5t = scr.tile([P, W * 5], F32, tag="t5" + sfx, name="t5" + sfx)
                t5 = t5t[:].rearrange("p (k c) -> p k c", c=5)
                vec.tensor_tensor(out=t5, in0=g5[:, :, 1, 0:5],
                                  in1=g5[:, :, 0, 0:5], op=AL.subtract)
                vec.tensor_tensor(out=t5, in0=t5, in1=sel5, op=AL.mult)
                vec.tensor_tensor(out=t5, in0=t5, in1=g5[:, :, 0, 0:5],
                                  op=AL.add)
                whr = t5[:, :, 2:4]
                cr = t5[:, :, 4]

                # signed sqrt of whr written back over whr -> t5[:, :, 0:4]
                # becomes (x_r, y_r, ssq(w_r), ssq(h_r))
                sq_ = t2("sq_")
                sg_ = t2("sg_")
                act.activation(sq_, whr, ACT.Abs)
                act.activation(sq_, sq_, ACT.Sqrt, bias=eps_t[:])
                act.activation(sg_, whr, ACT.Sign)
                vec.tensor_tensor(out=t5[:, :, 2:4], in0=sq_, in1=sg_,
                                  op=AL.mult)

                # coord+size: accum 5*sum((XYS4 - t5[:,:,0:4])^2)
                d4 = t4("d4")
                d4f = d4.rearrange("p k b r -> p k (b r)")
                vec.tensor_tensor(out=d4f, in0=XYS4, in1=t5[:, :, 0:4],
                                  op=AL.subtract)
                d4sq = t4("d4sq")
                act.activation(d4sq.rearrange("p k b r -> p k (b r)"), d4f,
                               ACT.Square, scale=RT5,
                               accum_out=acc[:, si:si + 1])

                # obj: accum 0.5*(cr-2)^2  (= (cr-1)^2 - 0.5cr^2 + 1)
                o1 = t1("o1")
                act.activation(o1, cr, ACT.Square, scale=RT2I, bias=nrt2_t[:],
                               accum_out=acc[:, 4 + si:5 + si])

                # class terms: accum sum(cls^2) and sum((h+cls)^2)
                clssq = scr.tile([P, W * 20], F32, tag="clssq" + sfx,
                                 name="clssq" + sfx)
                act.activation(clssq[:].rearrange("p (k c) -> p k c", c=20),
                               clsg, ACT.Square,
                               accum_out=acc[:, 8 + si:9 + si])
                big = scr.tile([P, W * 20], F32, tag="big" + sfx,
                               name="big" + sfx)
                big3 = big[:].rearrange("p (k c) -> p k c", c=20)
                vec.tensor_tensor(out=big3, in0=h3, in1=clsg, op=AL.add)
                hpc = scr.tile([P, W * 20], F32, tag="hpc" + sfx,
                               name="hpc" + sfx)
                act.activation(hpc[:].rearrange("p (k c) -> p k c", c=20),
                               big3, ACT.Square,
                               accum_out=acc[:, 12 + si:13 + si])

            for si in range(NG):
                sec_math(si)

            nc.sync.dma_start(out=out[:], in_=acc[:])
    nc.compile()
    return nc


def _get_nc():
    if "nc" not in _cache:
        _cache["nc"] = _build()
    return _cache["nc"]


def _host_prep(output, target):
    f32 = np.float32
    out_flat = output.reshape(CELLS, D)

    pt = np.zeros((CELLS, 32), dtype=f32)
    pt[:, :D] = out_flat
    conf_all = np.ascontiguousarray(out_flat[:, 4:10:5])

    bid = target[:, 7].astype(np.int64)
    gx = target[:, 4].astype(np.int64)
    gy = target[:, 5].astype(np.int64)
    cell = bid * (GRID * GRID) + gx * GRID + gy

    order = np.argsort(cell, kind="stable")
    ts = target[order]
    cs = cell[order]
    core = cs // CELLS_CORE
    wloc = (cs % CELLS_CORE) // WCELLS           # window 0/1
    mod = cs % 2                                 # parity within pair row
    grp = wloc * 2 + mod
    lp = ((cs % WCELLS) // 2).astype(np.int16)   # local pair row [0, 25088)

    x = ts[:, 0].astype(f32)
    y = ts[:, 1].astype(f32)
    w_ = ts[:, 2].astype(f32)
    h_ = ts[:, 3].astype(f32)
    c35 = f32(3.5)
    fields = np.empty((NTGT, 9), dtype=f32)
    fields[:, 0] = x
    fields[:, 1] = y
    fields[:, 2] = np.sign(w_) * np.sqrt(np.abs(w_) + f32(1e-6))   # ssqw
    fields[:, 3] = np.sign(h_) * np.sqrt(np.abs(h_) + f32(1e-6))   # ssqh
    fields[:, 4] = x - c35 * w_      # lef
    fields[:, 5] = y - c35 * h_      # top
    fields[:, 6] = x + c35 * w_      # rig
    fields[:, 7] = y + c35 * h_      # bot
    fields[:, 8] = (w_ * h_) * f32(49.0)
    clsid = ts[:, 6].astype(np.int64)
    hoh_all = np.zeros((NTGT, 20), dtype=f32)
    hoh_all[np.arange(NTGT), clsid] = f32(-2.0)

    in_maps = []
    for c in range(8):
        sel_c = core == c
        idxs = np.full(NS, ZROW, dtype=np.int16)
        fld = np.zeros((NS, 9), dtype=f32)
        hoh = np.zeros((NS, 20), dtype=f32)
        for gi in range(NG):
            selm = sel_c & (grp == gi)
            n = int(selm.sum())
            assert n <= CAP, f"group overflow: core {c} grp {gi} n={n}"
            s0 = gi * CAP
            idxs[s0:s0 + n] = lp[selm]
            fld[s0:s0 + n] = fields[selm]
            hoh[s0:s0 + n] = hoh_all[selm]

        idx16 = np.tile(idxs.reshape(IDXW, 16).T, (8, 1))          # [128, 576]
        fldf = np.empty((P, 9 * NCHUNK), dtype=f32)
        fldf[:, :4 * NCHUNK] = fld[:, 0:4].reshape(
            NCHUNK, P, 4).transpose(1, 0, 2).reshape(P, 4 * NCHUNK)
        fldf[:, 4 * NCHUNK:6 * NCHUNK] = fld[:, 4:6].reshape(
            NCHUNK, P, 2).transpose(1, 0, 2).reshape(P, 2 * NCHUNK)
        fldf[:, 6 * NCHUNK:8 * NCHUNK] = fld[:, 6:8].reshape(
            NCHUNK, P, 2).transpose(1, 0, 2).reshape(P, 2 * NCHUNK)
        fldf[:, 8 * NCHUNK:] = fld[:, 8].reshape(NCHUNK, P).T
        hcls = np.ascontiguousarray(
            hoh.reshape(NCHUNK, P, 20).transpose(1, 0, 2).reshape(P, 20 * NCHUNK))
        wslab = pt[c * CELLS_CORE:(c + 1) * CELLS_CORE].reshape(2, PR_WIN, 64)
        win = np.zeros((2 * WINROWS, 64), dtype=f32)
        win[:PR_WIN] = wslab[0]
        win[WINROWS:WINROWS + PR_WIN] = wslab[1]
        confc = np.ascontiguousarray(
            conf_all[c * CELLS_CORE:(c + 1) * CELLS_CORE]).reshape(P, CONF_W)
        in_maps.append({
            "win": win,
            "idx": np.ascontiguousarray(idx16),
            "fldf": fldf,
            "hcls": hcls,
            "conf": confc,
        })
    return in_maps


def _reduce(results):
    # cols 0-3: 5*coordsize; 4-7: 0.5*(cr-2)^2; 8-11: S_c = sum(cls^2);
    # 12-15: S_hpc = sum((h+cls)^2); 16: 0.5*conf^2.
    # loss = coordsize + obj + S_c + (S_hpc - S_c - 4*NTGT)/2 + conf
    #        - 2*n_pad (obj residue of padding) - 0 (real-slot +1's cancel)
    tot = 0.0
    for res in results:
        p = res["partial"].astype(np.float64)
        tot += float(p[:, 0:8].sum())                       # coordsize + obj
        tot += float((p[:, 8:12] + p[:, 12:16]).sum()) / 2  # (S_c + S_hpc)/2
        tot += float(p[:, 16].sum())                        # conf
    tot -= 2.0 * NTGT                  # polarization: -4*NTGT/2
    tot -= 2.0 * (8 * NS - NTGT)       # padding obj residue
    return np.float32(tot)


def run(output, target, trace=False, trace_cores=None):
    from concourse.bass_utils import run_bass_kernel_spmd

    nc = _get_nc()
    in_maps = _host_prep(np.asarray(output), np.asarray(target))
    r = run_bass_kernel_spmd(nc, in_maps, core_ids=list(range(8)), trace=trace,
                             trace_cores=trace_cores)
    return _reduce(r.results), r


def kernel(output, target):
    return run(output, target)[0]
